# revision 38
# baseline (speedup 1.0000x reference)
"""Trainium2 Bass kernel for nn_AttnBlock (B=16, C=512, H=W=32, T=180, G=32).

Math: the module broadcasts the text condition across channels, so the k/v
rows are identical for every channel and the attention collapses to rank-1:

  per batch b:
    group-norm stats over x[b]:  mu_g, rstd_g  (32 groups of 16 ch x 1024 px)
    a[c]  = colsum(wq)[c]*gamma[c]*rstd_{g(c)}
    s[n]  = sum_c a[c]*x[c,n] + const_b          (const_b folds mu/beta/bq)
    kap[f] = SCALE*(wk @ cond_b + bk)[f];  vb[f] = (wv @ cond_b + bv)[f]
    w[n]  = sum_f vb[f]*e^{s[n]*kap[f]} / sum_f e^{s[n]*kap[f]}
    out[c,n] = x[c,n] + rowsum(wo)[c]*w[n] + bo[c]

and w(s) is replaced by the degree-J Taylor rational
    w(s) ~= (sum_j m_j s^j/j!) / (sum_j mu_j s^j/j!),
    m_j = sum_f vb[f]*kap[f]^j,  mu_j = sum_f kap[f]^j
which is exact to ~1e-7 here (|s*kap| < 2.3), removing the F x N = 1M-point
exp/softmax entirely. Moments come on-device from kap/vb tiles; powers are
built log-depth.

Layout: per-pixel work runs n-major ([128, 8] tiles, n = 8*p + j): the
s-matvec emits n-major directly (x-chunks as the stationary operand, 32
column matmuls), so the rational evaluation is ~15 tiny ops instead of
128x-replicated row math; w returns to row layout via one 2KB SBUF DMA.
Output = PE outer-product (wo_sum (x) w + bo) + identity-matmul residual
accumulation in PSUM, drained by ACT copies / DVE adds, stored as bf16.

I/O is bf16 (x up-cast on host, out down-cast after the gather); weight-only
prepacking (colsum(wq)*gamma, rowsum(wo), wk^T*SCALE, layout packing, bf16
casts) is host-side; all data-dependent compute is on-device.

Sharding: data-parallel over batch, 2 batches per core, 8 cores, no
collectives.  Worst-case rel err vs the fp32 reference: 6.0e-3 on HW
(gate 2e-2).  HW exec time (identical-size-NEFF loop differencing):
~42 us/body vs 132.8 us for the previous exp-based kernel (~3.1x).

Hardware-validity notes learned the hard way (CoreSim accepts all of
these, the walrus BIR verifier does not):
  - an engine op may read at most ONE non-scalar operand from PSUM
  - all operands of an engine op must have the same AP rank (pad tiles so
    multi-slot outputs cannot be flattened)
  - broadcast (stride-0) AP dims must not be innermost
"""
import numpy as np
from contextlib import ExitStack

B, C, HW, N, T = 16, 512, 32, 1024, 180
F = 1024                      # in_features == H*W
G = 32                        # groups; 16 channels per group
NCORES, BPC = 8, 2            # cores, batches per core
NCH = C // 128                # 4 channel chunks
NFC = F // 128                # 8 feature chunks
NNB = N // 128                # 8 n blocks
EPS = 1e-6
SCALE = float(C) ** -0.5
J = 8                         # Taylor degree

_CACHE = {}


def _legalize_sync(nc, mybir):
    """This walrus build accepts at most one sync-wait command per
    instruction; hoist extra waits onto preceding same-engine NOPs."""
    k = 0
    for fn in nc.m.functions:
        for blk in fn.blocks:
            new = []
            for ins in blk.instructions:
                si = ins.sync_info
                if si is not None and si.on_wait is not None and len(si.on_wait) > 1:
                    for w in list(si.on_wait[:-1]):
                        nop = mybir.InstNoOp(name=f"syncsplit-{k}", ins=[], outs=[])
                        k += 1
                        nop.engine = ins.engine
                        nop.sync_info = mybir.SyncInfo(on_wait=[w], on_update=[])
                        new.append(nop)
                    ins.sync_info = mybir.SyncInfo(
                        on_wait=[si.on_wait[-1]],
                        on_update=list(si.on_update or []))
                new.append(ins)
            blk.instructions[:] = new


def _build(reps=1, legalize=True, loops=1):
    import concourse.bass as bass
    import concourse.mybir as mybir
    import concourse.tile as tile
    from concourse.tile import add_dep_helper

    f32 = mybir.dt.float32
    bf16 = mybir.dt.bfloat16
    Act = mybir.ActivationFunctionType
    Alu = mybir.AluOpType
    Ax = mybir.AxisListType

    nc = bass.Bass()

    x_d = nc.dram_tensor("x_sh", [BPC, C, N], bf16, kind="ExternalInput")
    condp_d = nc.dram_tensor("condp", [128, 2 * BPC], bf16, kind="ExternalInput")
    p128_d = nc.dram_tensor("p128f", [128, 46], f32, kind="ExternalInput")
    p8_d = nc.dram_tensor("p8f", [8, 132], f32, kind="ExternalInput")
    pbf_d = nc.dram_tensor("pbf", [128, 2176], bf16, kind="ExternalInput")
    p52_d = nc.dram_tensor("p52", [T - 128, 2 * F], bf16, kind="ExternalInput")
    wobo_d = nc.dram_tensor("wobo", [2, NCH, 128], bf16, kind="ExternalInput")
    out_d = nc.dram_tensor("out", [BPC, C, N], bf16, kind="ExternalOutput")

    NS = 2 * J + 1            # pow slots: [0..J-1]=kap^1..kap^J, [J..2J]=vb*kap^0..J

    with tile.TileContext(nc) as tc, ExitStack() as ctx:
        singles = ctx.enter_context(tc.tile_pool(name="singles", bufs=1))
        xpool = ctx.enter_context(tc.tile_pool(name="xpool", bufs=3))
        opool = ctx.enter_context(tc.tile_pool(name="opool", bufs=2))
        bpool = ctx.enter_context(tc.tile_pool(name="bpool", bufs=2))
        ps_sm = ctx.enter_context(tc.tile_pool(name="ps_sm", bufs=3, space="PSUM"))
        ps_s = ctx.enter_context(tc.tile_pool(name="ps_s", bufs=2, space="PSUM"))
        ps_out = ctx.enter_context(tc.tile_pool(name="ps_out", bufs=3, space="PSUM"))

        # ---- constants / one-time loads ------------------------------------
        ones_col = singles.tile([128, 1], f32)
        nc.vector.memset(ones_col, 1.0)
        ones8w = singles.tile([8, 128], f32)
        nc.gpsimd.memset(ones8w, 1.0)
        ones_row = singles.tile([1, 128], f32)
        nc.vector.memset(ones_row, 1.0)
        eps8 = singles.tile([8, 1], f32)
        nc.vector.memset(eps8, EPS)
        tl = singles.tile([1, 1], f32)

        w2t = [singles.tile([2, N], bf16, name=f"w2_{b}") for b in range(BPC)]
        for b in range(BPC):
            nc.gpsimd.memset(w2t[b], 1.0)      # row 0 overwritten per rep

        # prologue: 5 packed DMAs, emitted AFTER the x loads in single-rep
        # mode so x hits the (serial) DMA device first
        p128 = singles.tile([128, 46], f32)
        p8 = singles.tile([8, 132], f32)
        pbf = singles.tile([128, 2176], bf16)
        p52 = singles.tile([128, 2 * F], bf16)
        wobo = singles.tile([2, NCH, 128], bf16)

        def emit_prologue():
            nc.sync.dma_start(p128, p128_d[:, :])
            nc.sync.dma_start(p8, p8_d[:, :])
            nc.scalar.dma_start(pbf, pbf_d[:, :])
            nc.scalar.dma_start(p52[0:T - 128, :], p52_d[:, :])
            nc.scalar.dma_start(wobo, wobo_d[:, :, :])
            nc.scalar.activation(tl, eps8[0:1, 0:1], Act.Sqrt)  # table

        ind128 = p128[:, 0:8]
        wg = p128[:, 8:12]
        sconst = p128[:, 12:13]
        bkv = p128[:, 13:29].rearrange("p (q a) -> p q a", q=2)
        invf = p128[:, 29:46]
        indT8 = p8[:, 0:128]
        wgg = p8[:, 128:132]
        ident = pbf[:, 0:128]
        wkT0 = pbf[:, 128:1152]
        wvT0 = pbf[:, 1152:2176]
        wkT1 = p52[:, 0:F]
        wvT1 = p52[:, F:2 * F]

        rings = [nc.sync, nc.scalar]      # HWDGE rings; big DMAs alternate

        S = [dict() for _ in range(BPC)]

        def stage_load(b):
            ring = rings[b]
            if b == 0:
                condc = bpool.tile([128, 2 * BPC], bf16, tag="cond",
                                   name="cond")
                nc.scalar.dma_start(condc, condp_d[:, :])
                S[0]["cond"] = condc
                S[1]["cond"] = condc
            xh = [xpool.tile([128, 2, N], bf16, tag=f"xh{i}",
                             name=f"xh{b}{i}") for i in range(2)]
            if b == 0:
                # small first pieces on SP; second half on the ACT ring so it
                # reaches the (serial) DMA device before x_b1
                ring.dma_start(
                    xh[0][:, 0, :],
                    x_d[0, 0:128].rearrange("(a p) n -> p (a n)", p=128))
                ring.dma_start(
                    xh[0][:, 1, :],
                    x_d[0, 128:256].rearrange("(a p) n -> p (a n)", p=128))
                nc.scalar.dma_start(
                    xh[1],
                    x_d[0, 256:512].rearrange("(a p) n -> p a n", p=128))
            else:
                ring.dma_start(
                    xh[0],
                    x_d[b, 0:256].rearrange("(a p) n -> p a n", p=128))
                ring.dma_start(
                    xh[1],
                    x_d[b, 256:512].rearrange("(a p) n -> p a n", p=128))
            S[b]["xh"] = xh

        def stage_stats(b):
            xh = S[b]["xh"]
            mv = bpool.tile([128, NCH, 2], f32, tag="mv", name=f"mv{b}")
            prev = None
            prot = S[0].get("prot", []) if b == 1 else []
            for a in range(NCH):
                xa = xh[a // 2][:, a % 2, :]
                st = bpool.tile([128, 2, 6], f32, tag="st", name=f"st{b}{a}")
                i1 = nc.vector.bn_stats(st[:, 0, :], xa[:, 0:512])
                # keep the DVE wait-queue from letting streaming bn_stats
                # starve the tiny chain ops (they miss their slot by ~30ns)
                nc.vector.bn_stats(st[:, 1, :], xa[:, 512:1024])
                prev = nc.vector.bn_aggr(mv[:, a, :], st)
            mv2 = bpool.tile([128, NCH, 2], f32, tag="mv2", name=f"mv2{b}")
            msq = bpool.tile([128, NCH], f32, tag="msq", name=f"msq{b}")
            nc.gpsimd.tensor_mul(msq, mv[:, :, 0], mv[:, :, 0])
            nc.gpsimd.tensor_copy(mv2[:, :, 0], mv[:, :, 0])
            i_mv2 = nc.gpsimd.tensor_add(mv2[:, :, 1], mv[:, :, 1], msq)
            if b == 0:
                S[0]["prot"] = [i_mv2]
            gs_ps = ps_sm.tile([8, NCH, 2], f32, tag="sm", name=f"gs{b}")
            nc.tensor.matmul(gs_ps, ind128, mv2.rearrange("p a c -> p (a c)"),
                             start=True, stop=True)
            gsb = bpool.tile([8, NCH, 2], f32, tag="gsb", name=f"gsb{b}")
            nc.scalar.copy(gsb, gs_ps)
            msqg = bpool.tile([8, NCH], f32, tag="msqg", name=f"msqg{b}")
            nc.vector.tensor_mul(msqg, gsb[:, :, 0], gsb[:, :, 0])
            varg = bpool.tile([8, NCH], f32, tag="varg", name=f"varg{b}")
            nc.vector.tensor_sub(varg, gsb[:, :, 1], msqg)
            sd = bpool.tile([8, NCH], f32, tag="sd", name=f"sd{b}")
            nc.scalar.activation(sd, varg, Act.Sqrt, bias=eps8[:, 0:1])
            rm = bpool.tile([8, 2, NCH], f32, tag="rm", name=f"rm{b}")
            nc.vector.reciprocal(rm[:, 0, :], sd)
            i_rm1 = nc.vector.tensor_mul(rm[:, 1, :], gsb[:, :, 0],
                                         rm[:, 0, :])
            if b == 0:
                S[0]["prot"].append(i_rm1)
            rep_ps = ps_sm.tile([128, 8], f32, tag="sm", name=f"rep{b}")
            nc.tensor.matmul(rep_ps, indT8, rm.rearrange("g x a -> g (x a)"),
                             start=True, stop=True)
            a_all = bpool.tile([128, NCH], bf16, tag="a_all", name=f"a{b}")
            i_aall = nc.vector.tensor_mul(a_all, wg, rep_ps[:, 0:NCH])
            if b == 0:
                S[0]["prot"].append(i_aall)
            wmg = bpool.tile([8, NCH], f32, tag="wmg", name=f"wmg{b}")
            nc.vector.tensor_mul(wmg, wgg, rm[:, 1, :])
            wms_ps = ps_sm.tile([128, NCH], f32, tag="sm", name=f"wms{b}")
            nc.tensor.matmul(wms_ps, ones8w, wmg, start=True, stop=True)
            wsum = bpool.tile([128, 1], f32, tag="wsum", name=f"wsum{b}")
            nc.vector.tensor_reduce(wsum, wms_ps, axis=Ax.X, op=Alu.add)
            constb = bpool.tile([128, 1], f32, tag="constb", name=f"cb{b}")
            i_cb = nc.vector.tensor_sub(constb, sconst, wsum)
            if b == 0:
                S[0]["prot"].append(i_cb)
            S[b]["a_all"], S[b]["constb"] = a_all, constb

        def stage_kv(b):
            cond = S[b]["cond"]
            c0, c1 = cond[:, 2 * b:2 * b + 1], cond[0:T - 128,
                                                    2 * b + 1:2 * b + 2]
            kv_ps = ps_sm.tile([128, 2 * NFC], f32, tag="sm", name=f"kv{b}")
            for kvi, (t0, t1) in enumerate(((wkT0, wkT1), (wvT0, wvT1))):
                for fc in range(NFC):
                    o = kv_ps[:, kvi * NFC + fc:kvi * NFC + fc + 1]
                    nc.tensor.matmul(o, t0[:, 128 * fc:128 * (fc + 1)], c0,
                                     start=True, stop=False,
                                     skip_group_check=True)
                    nc.tensor.matmul(o, t1[0:T - 128, 128 * fc:128 * (fc + 1)],
                                     c1, start=False,
                                     stop=True, skip_group_check=True)
            # padded slot rows ([128, NS, NFC+1]) keep every multi-slot op
            # 3-dim on ALL operands (the walrus verifier rejects mixed-rank
            # operand lists); pad col is zeroed so the moment matmul can
            # read the tile flat.
            powt = bpool.tile([128, NS, NFC + 1], f32, tag="pow",
                              name=f"pow{b}")
            nc.gpsimd.memset(powt[:, :, NFC:NFC + 1], 0.0)
            nc.vector.tensor_add(powt[:, 0, 0:NFC], kv_ps[:, 0:NFC],
                                 bkv[:, 0, :])
            nc.vector.tensor_add(powt[:, J, 0:NFC], kv_ps[:, NFC:2 * NFC],
                                 bkv[:, 1, :])
            # log-depth power build: slots 0..J-1 = kap^1..kap^J
            def bcast(src_ap, nrep):
                return bass.AP(tensor=src_ap.tensor, offset=src_ap.offset,
                               ap=[list(src_ap.ap[0]), [0, nrep], [1, NFC]])
            nc.gpsimd.tensor_mul(powt[:, 1, 0:NFC], powt[:, 0, 0:NFC],
                                 powt[:, 0, 0:NFC])
            nc.gpsimd.tensor_tensor(powt[:, 2:4, 0:NFC], powt[:, 0:2, 0:NFC],
                                    bcast(powt[:, 1, 0:NFC], 2), Alu.mult)
            nc.gpsimd.tensor_tensor(powt[:, 4:8, 0:NFC], powt[:, 0:4, 0:NFC],
                                    bcast(powt[:, 3, 0:NFC], 4), Alu.mult)
            nc.gpsimd.tensor_tensor(powt[:, J + 1:2 * J + 1, 0:NFC],
                                    powt[:, 0:J, 0:NFC],
                                    bcast(powt[:, J, 0:NFC], J), Alu.mult)
            S[b]["powt"] = powt

        def stage_smv(b):
            a_all, xh = S[b]["a_all"], S[b]["xh"]
            s_ps = ps_s.tile([128, NNB], f32, tag="s", name=f"s{b}")
            for a in range(NCH):
                xt = xh[a // 2]
                for jb in range(NNB):
                    lhsT = bass.AP(
                        tensor=xt.tensor,
                        offset=xt.offset + (a % 2) * N + jb,
                        ap=[list(xt.ap[0]), [NNB, 128]])
                    nc.tensor.matmul(s_ps[:, jb:jb + 1], lhsT,
                                     a_all[:, a:a + 1], start=(a == 0),
                                     stop=(a == NCH - 1),
                                     skip_group_check=True)
            S[b]["s_ps"] = s_ps

        def stage_mom(b):
            powt = S[b]["powt"]
            mom_ps = ps_sm.tile([1, NS * (NFC + 1)], f32, tag="sm",
                                name=f"mom{b}")
            nc.tensor.matmul(mom_ps, ones_col,
                             powt.rearrange("p s a -> p (s a)"),
                             start=True, stop=True)
            coefs = bpool.tile([1, NS], f32, tag="coefs", name=f"coefs{b}")
            nc.vector.tensor_reduce(
                coefs, mom_ps.rearrange("o (s a) -> o s a", s=NS),
                axis=Ax.X, op=Alu.add)
            crep_ps = ps_sm.tile([128, NS], f32, tag="sm", name=f"crep{b}")
            nc.tensor.matmul(crep_ps, ones_row, coefs, start=True, stop=True)
            crep = bpool.tile([128, NS], f32, tag="crep", name=f"crep{b}")
            nc.vector.tensor_mul(crep, crep_ps, invf)
            S[b]["crep"] = crep

        def stage_eval(b):
            s_ps, constb, crep = S[b]["s_ps"], S[b]["constb"], S[b]["crep"]
            # slot-major storage: spow[:, j, :] = s^(j+1); every engine WRITE
            # is innermost-contiguous (strided-innermost outputs fail the
            # walrus BIR verifier); the dot reads it via a transposed AP.
            spow = bpool.tile([128, J, NNB + 1], f32, tag="spow",
                              name=f"spow{b}")
            nc.vector.tensor_scalar_add(spow[:, 0, 0:NNB], s_ps, constb)

            def sbc(j0, brep):
                a = spow[:, j0, 0:NNB]
                return bass.AP(tensor=a.tensor, offset=a.offset,
                               ap=[list(a.ap[0]), [0, brep], [1, NNB]])
            nc.gpsimd.tensor_mul(spow[:, 1, 0:NNB], spow[:, 0, 0:NNB],
                                 spow[:, 0, 0:NNB])
            nc.gpsimd.tensor_tensor(spow[:, 2:4, 0:NNB], spow[:, 0:2, 0:NNB],
                                    sbc(1, 2), Alu.mult)
            nc.gpsimd.tensor_tensor(spow[:, 4:8, 0:NNB], spow[:, 0:4, 0:NNB],
                                    sbc(3, 4), Alu.mult)
            spow_nm = bass.AP(tensor=spow.tensor, offset=spow.offset,
                              ap=[list(spow.ap[0]), [1, NNB], [NNB + 1, J]])
            pp = bpool.tile([128, 2, NNB, J + 1], f32, tag="pp",
                            name=f"pp{b}")
            dn = bpool.tile([128, 2, NNB], f32, tag="dn", name=f"dn{b}")

            def dot(which, eng, ccols, c0_ap, c0_imm):
                cb = bass.AP(tensor=crep.tensor,
                             offset=crep.offset + ccols,
                             ap=[list(crep.ap[0]), [0, NNB], [1, J]])
                eng.tensor_tensor(pp[:, which, :, 0:J], spow_nm, cb, Alu.mult)
                nc.vector.tensor_reduce(dn[:, which, :],
                                        pp[:, which, :, 0:J],
                                        axis=Ax.X, op=Alu.add)
                if c0_ap is not None:
                    eng.tensor_scalar_add(dn[:, which, :],
                                          dn[:, which, :], c0_ap)
                else:
                    nc.vector.tensor_scalar_add(dn[:, which, :],
                                                dn[:, which, :], c0_imm)

            dot(1, nc.vector, 0, None, float(F))           # denominator
            dot(0, nc.gpsimd if b == 0 else nc.vector,
                J + 1, crep[:, J:J + 1], None)             # numerator
            rd = bpool.tile([128, NNB], f32, tag="rd", name=f"rd{b}")
            nc.vector.reciprocal(rd, dn[:, 1, :])
            w_nm = bpool.tile([128, NNB], bf16, tag="w_nm", name=f"w{b}")
            nc.vector.tensor_mul(w_nm, dn[:, 0, :], rd)
            # scatter n-major w back to a row: w2[0, n] = w_nm[n//8, n%8]
            # (DMA pairs elements in flatten order; 16B descriptors)
            rings[b].dma_start(w2t[b][0:1, :], w_nm)
            S[b]["w2"] = w2t[b]

        def stage_out(b):
            xh, w2 = S[b]["xh"], S[b]["w2"]
            o_sb = opool.tile([128, NCH, N], bf16, tag="o", name=f"o{b}")
            for a in range(NCH):
                for h in range(2):
                    dve = (h == 1)
                    o_ps = ps_out.tile([128, 512], f32, tag="o",
                                       name=f"ops{b}{a}{h}")
                    nc.tensor.matmul(o_ps, wobo[:, a, :],
                                     w2[:, 512 * h:512 * (h + 1)],
                                     start=True, stop=dve,
                                     skip_group_check=True)
                    xa = xh[a // 2][:, a % 2, :]
                    dst = o_sb[:, a, 512 * h:512 * (h + 1)]
                    if dve:
                        nc.vector.tensor_add(
                            dst, o_ps, xa[:, 512 * h:512 * (h + 1)])
                    else:
                        nc.tensor.matmul(o_ps, ident,
                                         xa[:, 512 * h:512 * (h + 1)],
                                         start=False, stop=True,
                                         skip_group_check=True)
                        nc.scalar.copy(dst, o_ps)
                rings[b].dma_start(out_d[b, 128 * a:128 * (a + 1), :],
                                   o_sb[:, a, :])

        def emit_rep():
            stage_load(0)
            stage_load(1)
            if S[0].get("first", True):
                emit_prologue()
                S[0]["first"] = False
            stage_stats(0)
            stage_kv(0)
            stage_smv(0)
            stage_mom(0)
            stage_eval(0)
            stage_out(0)
            stage_stats(1)
            stage_kv(1)
            stage_smv(1)
            stage_mom(1)
            stage_eval(1)
            stage_out(1)

        if loops == 1:
            for _ in range(reps):
                emit_rep()
        else:
            emit_prologue()
            S[0]["first"] = False
            with tc.For_i(0, loops):
                for _ in range(reps):
                    emit_rep()

    if legalize:
        _legalize_sync(nc, mybir)
    return nc


def _indicators():
    ind128 = np.zeros((128, 8), np.float32)
    indT8 = np.zeros((8, 128), np.float32)
    for g in range(8):
        ind128[16 * g:16 * g + 16, g] = 1.0 / 16.0
        indT8[g, 16 * g:16 * g + 16] = 1.0
    return ind128, indT8


def _host_pack(inputs):
    """Weight-only prepacking + dtype/layout prep shared by all cores."""
    import ml_dtypes
    import math
    bf = ml_dtypes.bfloat16
    f = {k: np.asarray(v, dtype=np.float32) for k, v in inputs.items()}
    wqc = f["wq"].sum(axis=0)                       # colsum over out channels
    wg = (wqc * f["gamma"]).astype(np.float32)      # [C]
    sconst = np.float32(f["bq"].sum() + (wqc * f["beta"]).sum())
    wkTs = (f["wk"].T * SCALE).astype(bf)           # [T, F]
    wvT = f["wv"].T.astype(bf)
    bks = (f["bk"] * SCALE).astype(np.float32)
    bkv = np.stack([bks.reshape(NFC, 128).T,
                    f["bv"].astype(np.float32).reshape(NFC, 128).T],
                   axis=1)                          # [128, 2, NFC]
    wos = f["wo"].sum(axis=1)
    wobo = np.stack([wos.reshape(NCH, 128),
                     f["bo"].reshape(NCH, 128)], axis=0).astype(bf)
    invf = np.array([1.0 / math.factorial(j) for j in range(1, J + 1)] +
                    [1.0 / math.factorial(j) for j in range(J + 1)],
                    np.float32)
    ind128, indT8 = _indicators()

    p128 = np.zeros((128, 46), np.float32)
    p128[:, 0:8] = ind128
    p128[:, 8:12] = wg.reshape(NCH, 128).T
    p128[:, 12] = sconst
    p128[:, 13:29] = bkv.reshape(128, 16)
    p128[:, 29:46] = invf[None, :]
    p8 = np.zeros((8, 132), np.float32)
    p8[:, 0:128] = indT8
    p8[:, 128:132] = wg.reshape(G, C // G).sum(1).reshape(NCH, 8).T
    pbf = np.zeros((128, 2176), bf)
    pbf[:, 0:128] = np.eye(128, dtype=bf)
    pbf[:, 128:1152] = wkTs[0:128]
    pbf[:, 1152:2176] = wvT[0:128]
    p52 = np.zeros((T - 128, 2 * F), bf)
    p52[:, 0:F] = wkTs[128:T]
    p52[:, F:2 * F] = wvT[128:T]
    shared = {
        "p128f": p128, "p8f": p8, "pbf": pbf, "p52": p52,
        "wobo": np.ascontiguousarray(wobo),
    }
    x_bf = np.asarray(f["x"], np.float32).reshape(B, C, N).astype(bf)
    condp = np.zeros((128, 2 * B), bf)
    condT = f["condition"].T.astype(bf)             # [T, B]
    for b in range(B):
        condp[:, 2 * b] = condT[0:128, b]
        condp[0:T - 128, 2 * b + 1] = condT[128:T, b]
    return shared, x_bf, condp


def _dev_inputs(inputs, core_idx, _packed=None):
    shared, x_bf, x_bf_cond = (_packed if _packed is not None
                               else _host_pack(inputs))
    m = dict(shared)
    m["x_sh"] = np.ascontiguousarray(x_bf[BPC * core_idx:BPC * (core_idx + 1)])
    m["condp"] = np.ascontiguousarray(
        x_bf_cond[:, 2 * BPC * core_idx:2 * BPC * (core_idx + 1)])
    return m


def kernel(**inputs):
    from concourse.bass_utils import run_bass_kernel_spmd

    if "nc" not in _CACHE:
        _CACHE["nc"] = _build()
    nc = _CACHE["nc"]

    packed = _host_pack(inputs)
    in_maps = [_dev_inputs(inputs, i, packed) for i in range(NCORES)]
    res = run_bass_kernel_spmd(nc, in_maps, core_ids=list(range(NCORES)))
    out = np.concatenate([np.asarray(r["out"], dtype=np.float32)
                          for r in res.results], axis=0)
    return out.reshape(B, C, HW, HW)


# revision 41
# speedup vs baseline: 1.0903x; 1.0903x over previous
"""Trainium2 Bass kernel for nn_AttnBlock (B=16, C=512, H=W=32, T=180, G=32).

Math: the module broadcasts the text condition across channels, so the k/v
rows are identical for every channel and the attention collapses to rank-1:

  per batch b:
    group-norm stats over x[b]:  mu_g, rstd_g  (32 groups of 16 ch x 1024 px)
    a[c]  = colsum(wq)[c]*gamma[c]*rstd_{g(c)}
    s[n]  = sum_c a[c]*x[c,n] + const_b          (const_b folds mu/beta/bq)
    kap[f] = SCALE*(wk @ cond_b + bk)[f];  vb[f] = (wv @ cond_b + bv)[f]
    w[n]  = sum_f vb[f]*e^{s[n]*kap[f]} / sum_f e^{s[n]*kap[f]}
    out[c,n] = x[c,n] + rowsum(wo)[c]*w[n] + bo[c]

and w(s) is replaced by the degree-J Taylor rational
    w(s) ~= (sum_j m_j s^j/j!) / (sum_j mu_j s^j/j!),
    m_j = sum_f vb[f]*kap[f]^j,  mu_j = sum_f kap[f]^j
which is exact to ~1e-7 here (|s*kap| < 2.3), removing the F x N = 1M-point
exp/softmax entirely. Moments come on-device from kap/vb tiles; powers are
built log-depth.

Layout: per-pixel work runs n-major ([128, 8] tiles, n = 8*p + j): the
s-matvec emits n-major directly (x-chunks as the stationary operand, 32
column matmuls), so the rational evaluation is ~15 tiny ops instead of
128x-replicated row math; w returns to row layout via one 2KB SBUF DMA.
Output = PE outer-product (wo_sum (x) w + bo) + identity-matmul residual
accumulation in PSUM, drained by ACT copies / DVE adds, stored as bf16.

I/O is bf16 (x up-cast on host, out down-cast after the gather); weight-only
prepacking (colsum(wq)*gamma, rowsum(wo), wk^T*SCALE, layout packing, bf16
casts) is host-side; all data-dependent compute is on-device.

Sharding: data-parallel over batch, 2 batches per core, 8 cores, no
collectives.  Worst-case rel err vs the fp32 reference: 6.0e-3 on HW
(gate 2e-2).  HW exec time (identical-size-NEFF loop differencing,
cross-run drift ~+-3 us): ~44-49 us/body vs 132.8 us for the previous
exp-based kernel (~2.8-3x); TimelineSim cost model: 35.0 us vs 77.9 us.

Hardware-validity notes learned the hard way (CoreSim accepts all of
these, the walrus BIR verifier does not):
  - an engine op may read at most ONE non-scalar operand from PSUM
  - all operands of an engine op must have the same AP rank (pad tiles so
    multi-slot outputs cannot be flattened)
  - broadcast (stride-0) AP dims must not be innermost
"""
import numpy as np
from contextlib import ExitStack

B, C, HW, N, T = 16, 512, 32, 1024, 180
F = 1024                      # in_features == H*W
G = 32                        # groups; 16 channels per group
NCORES, BPC = 8, 2            # cores, batches per core
NCH = C // 128                # 4 channel chunks
NFC = F // 128                # 8 feature chunks
NNB = N // 128                # 8 n blocks
EPS = 1e-6
SCALE = float(C) ** -0.5
J = 8                         # Taylor degree

_CACHE = {}


def _legalize_sync(nc, mybir):
    """This walrus build accepts at most one sync-wait command per
    instruction; hoist extra waits onto preceding same-engine NOPs."""
    k = 0
    for fn in nc.m.functions:
        for blk in fn.blocks:
            new = []
            for ins in blk.instructions:
                si = ins.sync_info
                if si is not None and si.on_wait is not None and len(si.on_wait) > 1:
                    for w in list(si.on_wait[:-1]):
                        nop = mybir.InstNoOp(name=f"syncsplit-{k}", ins=[], outs=[])
                        k += 1
                        nop.engine = ins.engine
                        nop.sync_info = mybir.SyncInfo(on_wait=[w], on_update=[])
                        new.append(nop)
                    ins.sync_info = mybir.SyncInfo(
                        on_wait=[si.on_wait[-1]],
                        on_update=list(si.on_update or []))
                new.append(ins)
            blk.instructions[:] = new


def _build(reps=1, legalize=True, loops=1):
    import concourse.bass as bass
    import concourse.mybir as mybir
    import concourse.tile as tile
    from concourse.tile import add_dep_helper

    f32 = mybir.dt.float32
    bf16 = mybir.dt.bfloat16
    Act = mybir.ActivationFunctionType
    Alu = mybir.AluOpType
    Ax = mybir.AxisListType

    nc = bass.Bass()

    x_d = nc.dram_tensor("x_sh", [BPC, C, N], bf16, kind="ExternalInput")
    condp_d = nc.dram_tensor("condp", [128, 2 * BPC], bf16, kind="ExternalInput")
    p128_d = nc.dram_tensor("p128f", [128, 46], f32, kind="ExternalInput")
    p8_d = nc.dram_tensor("p8f", [8, 132], f32, kind="ExternalInput")
    pbf_d = nc.dram_tensor("pbf", [128, 2176], bf16, kind="ExternalInput")
    p52_d = nc.dram_tensor("p52", [T - 128, 2 * F], bf16, kind="ExternalInput")
    wobo_d = nc.dram_tensor("wobo", [2, NCH, 128], bf16, kind="ExternalInput")
    out_d = nc.dram_tensor("out", [BPC, C, N], bf16, kind="ExternalOutput")

    NS = 2 * J + 1            # pow slots: [0..J-1]=kap^1..kap^J, [J..2J]=vb*kap^0..J

    with tile.TileContext(nc) as tc, ExitStack() as ctx:
        singles = ctx.enter_context(tc.tile_pool(name="singles", bufs=1))
        xpool = ctx.enter_context(tc.tile_pool(name="xpool", bufs=3))
        opool = ctx.enter_context(tc.tile_pool(name="opool", bufs=2))
        bpool = ctx.enter_context(tc.tile_pool(name="bpool", bufs=2))
        ps_sm = ctx.enter_context(tc.tile_pool(name="ps_sm", bufs=3, space="PSUM"))
        ps_s = ctx.enter_context(tc.tile_pool(name="ps_s", bufs=1, space="PSUM"))
        ps_out = ctx.enter_context(tc.tile_pool(name="ps_out", bufs=4, space="PSUM"))

        # ---- constants / one-time loads ------------------------------------
        ones_col = singles.tile([128, 1], f32)
        nc.vector.memset(ones_col, 1.0)
        ones8w = singles.tile([8, 128], f32)
        nc.gpsimd.memset(ones8w, 1.0)
        ones_row = singles.tile([1, 128], f32)
        nc.vector.memset(ones_row, 1.0)
        eps8 = singles.tile([8, 1], f32)
        nc.vector.memset(eps8, EPS)
        tl = singles.tile([1, 1], f32)

        w2t = [singles.tile([2, N], bf16, name=f"w2_{b}") for b in range(BPC)]
        for b in range(BPC):
            nc.gpsimd.memset(w2t[b], 1.0)      # row 0 overwritten per rep

        # prologue: 5 packed DMAs, emitted AFTER the x loads in single-rep
        # mode so x hits the (serial) DMA device first
        p128 = singles.tile([128, 46], f32)
        p8 = singles.tile([8, 132], f32)
        pbf = singles.tile([128, 2176], bf16)
        p52 = singles.tile([128, 2 * F], bf16)
        wobo = singles.tile([2, NCH, 128], bf16)

        def emit_prologue():
            nc.sync.dma_start(p128, p128_d[:, :])
            nc.sync.dma_start(p8, p8_d[:, :])
            nc.scalar.dma_start(pbf, pbf_d[:, :])
            nc.scalar.dma_start(p52[0:T - 128, :], p52_d[:, :])
            nc.scalar.dma_start(wobo, wobo_d[:, :, :])
            nc.scalar.activation(tl, eps8[0:1, 0:1], Act.Sqrt)  # table

        ind128 = p128[:, 0:8]
        wg = p128[:, 8:12]
        sconst = p128[:, 12:13]
        bkv = p128[:, 13:29].rearrange("p (q a) -> p q a", q=2)
        invf = p128[:, 29:46]
        indT8 = p8[:, 0:128]
        wgg = p8[:, 128:132]
        ident = pbf[:, 0:128]
        wkT0 = pbf[:, 128:1152]
        wvT0 = pbf[:, 1152:2176]
        wkT1 = p52[:, 0:F]
        wvT1 = p52[:, F:2 * F]

        rings = [nc.sync, nc.scalar]      # HWDGE rings; big DMAs alternate

        S = [dict() for _ in range(BPC)]

        def stage_load(b):
            ring = rings[b]
            if b == 0:
                condc = bpool.tile([128, 2 * BPC], bf16, tag="cond",
                                   name="cond")
                nc.scalar.dma_start(condc, condp_d[:, :])
                S[0]["cond"] = condc
                S[1]["cond"] = condc
            xh = [xpool.tile([128, 2, N], bf16, tag=f"xh{i}",
                             name=f"xh{b}{i}") for i in range(2)]
            if b == 0:
                # small first pieces on SP; second half on the ACT ring so it
                # reaches the (serial) DMA device before x_b1
                ring.dma_start(
                    xh[0][:, 0, :],
                    x_d[0, 0:128].rearrange("(a p) n -> p (a n)", p=128))
                ring.dma_start(
                    xh[0][:, 1, :],
                    x_d[0, 128:256].rearrange("(a p) n -> p (a n)", p=128))
                nc.scalar.dma_start(
                    xh[1],
                    x_d[0, 256:512].rearrange("(a p) n -> p a n", p=128))
            else:
                ring.dma_start(
                    xh[0],
                    x_d[b, 0:256].rearrange("(a p) n -> p a n", p=128))
                ring.dma_start(
                    xh[1],
                    x_d[b, 256:512].rearrange("(a p) n -> p a n", p=128))
            S[b]["xh"] = xh

        def stage_stats(b):
            xh = S[b]["xh"]
            mv = bpool.tile([128, NCH, 2], f32, tag="mv", name=f"mv{b}")
            prev = None
            prot = S[0].get("prot", []) if b == 1 else []
            for a in range(NCH):
                xa = xh[a // 2][:, a % 2, :]
                st = bpool.tile([128, 2, 6], f32, tag="st", name=f"st{b}{a}")
                i1 = nc.vector.bn_stats(st[:, 0, :], xa[:, 0:512])
                # keep the DVE wait-queue from letting streaming bn_stats
                # starve the tiny chain ops (they miss their slot by ~30ns)
                nc.vector.bn_stats(st[:, 1, :], xa[:, 512:1024])
                prev = nc.vector.bn_aggr(mv[:, a, :], st)
            mv2 = bpool.tile([128, NCH, 2], f32, tag="mv2", name=f"mv2{b}")
            msq = bpool.tile([128, NCH], f32, tag="msq", name=f"msq{b}")
            nc.gpsimd.tensor_mul(msq, mv[:, :, 0], mv[:, :, 0])
            nc.gpsimd.tensor_copy(mv2[:, :, 0], mv[:, :, 0])
            i_mv2 = nc.gpsimd.tensor_add(mv2[:, :, 1], mv[:, :, 1], msq)
            if b == 0:
                S[0]["prot"] = [i_mv2]
            gs_ps = ps_sm.tile([8, NCH, 2], f32, tag="sm", name=f"gs{b}")
            nc.tensor.matmul(gs_ps, ind128, mv2.rearrange("p a c -> p (a c)"),
                             start=True, stop=True)
            gsb = bpool.tile([8, NCH, 2], f32, tag="gsb", name=f"gsb{b}")
            nc.scalar.copy(gsb, gs_ps)
            msqg = bpool.tile([8, NCH], f32, tag="msqg", name=f"msqg{b}")
            nc.vector.tensor_mul(msqg, gsb[:, :, 0], gsb[:, :, 0])
            varg = bpool.tile([8, NCH], f32, tag="varg", name=f"varg{b}")
            nc.vector.tensor_sub(varg, gsb[:, :, 1], msqg)
            sd = bpool.tile([8, NCH], f32, tag="sd", name=f"sd{b}")
            nc.scalar.activation(sd, varg, Act.Sqrt, bias=eps8[:, 0:1])
            rm = bpool.tile([8, 2, NCH], f32, tag="rm", name=f"rm{b}")
            nc.vector.reciprocal(rm[:, 0, :], sd)
            i_rm1 = nc.vector.tensor_mul(rm[:, 1, :], gsb[:, :, 0],
                                         rm[:, 0, :])
            if b == 0:
                S[0]["prot"].append(i_rm1)
            rep_ps = ps_sm.tile([128, 8], f32, tag="sm", name=f"rep{b}")
            nc.tensor.matmul(rep_ps, indT8, rm.rearrange("g x a -> g (x a)"),
                             start=True, stop=True)
            a_all = bpool.tile([128, NCH], bf16, tag="a_all", name=f"a{b}")
            i_aall = nc.vector.tensor_mul(a_all, wg, rep_ps[:, 0:NCH])
            if b == 0:
                S[0]["prot"].append(i_aall)
            wmg = bpool.tile([8, NCH], f32, tag="wmg", name=f"wmg{b}")
            nc.vector.tensor_mul(wmg, wgg, rm[:, 1, :])
            wms_ps = ps_sm.tile([128, NCH], f32, tag="sm", name=f"wms{b}")
            nc.tensor.matmul(wms_ps, ones8w, wmg, start=True, stop=True)
            wsum = bpool.tile([128, 1], f32, tag="wsum", name=f"wsum{b}")
            nc.vector.tensor_reduce(wsum, wms_ps, axis=Ax.X, op=Alu.add)
            constb = bpool.tile([128, 1], f32, tag="constb", name=f"cb{b}")
            i_cb = nc.vector.tensor_sub(constb, sconst, wsum)
            if b == 0:
                S[0]["prot"].append(i_cb)
            S[b]["a_all"], S[b]["constb"] = a_all, constb

        def stage_kv(b):
            cond = S[b]["cond"]
            c0, c1 = cond[:, 2 * b:2 * b + 1], cond[0:T - 128,
                                                    2 * b + 1:2 * b + 2]
            kv_ps = ps_sm.tile([128, 2 * NFC], f32, tag="sm", name=f"kv{b}")
            for kvi, (t0, t1) in enumerate(((wkT0, wkT1), (wvT0, wvT1))):
                for fc in range(NFC):
                    o = kv_ps[:, kvi * NFC + fc:kvi * NFC + fc + 1]
                    nc.tensor.matmul(o, t0[:, 128 * fc:128 * (fc + 1)], c0,
                                     start=True, stop=False,
                                     skip_group_check=True)
                    nc.tensor.matmul(o, t1[0:T - 128, 128 * fc:128 * (fc + 1)],
                                     c1, start=False,
                                     stop=True, skip_group_check=True)
            # padded slot rows ([128, NS, NFC+1]) keep every multi-slot op
            # 3-dim on ALL operands (the walrus verifier rejects mixed-rank
            # operand lists); pad col is zeroed so the moment matmul can
            # read the tile flat.
            powt = bpool.tile([128, NS, NFC + 1], f32, tag="pow",
                              name=f"pow{b}")
            nc.gpsimd.memset(powt[:, :, NFC:NFC + 1], 0.0)
            nc.vector.tensor_add(powt[:, 0, 0:NFC], kv_ps[:, 0:NFC],
                                 bkv[:, 0, :])
            nc.vector.tensor_add(powt[:, J, 0:NFC], kv_ps[:, NFC:2 * NFC],
                                 bkv[:, 1, :])
            # log-depth power build: slots 0..J-1 = kap^1..kap^J
            def bcast(src_ap, nrep):
                return bass.AP(tensor=src_ap.tensor, offset=src_ap.offset,
                               ap=[list(src_ap.ap[0]), [0, nrep], [1, NFC]])
            nc.gpsimd.tensor_mul(powt[:, 1, 0:NFC], powt[:, 0, 0:NFC],
                                 powt[:, 0, 0:NFC])
            nc.gpsimd.tensor_tensor(powt[:, 2:4, 0:NFC], powt[:, 0:2, 0:NFC],
                                    bcast(powt[:, 1, 0:NFC], 2), Alu.mult)
            nc.gpsimd.tensor_tensor(powt[:, 4:8, 0:NFC], powt[:, 0:4, 0:NFC],
                                    bcast(powt[:, 3, 0:NFC], 4), Alu.mult)
            nc.gpsimd.tensor_tensor(powt[:, J + 1:2 * J + 1, 0:NFC],
                                    powt[:, 0:J, 0:NFC],
                                    bcast(powt[:, J, 0:NFC], J), Alu.mult)
            S[b]["powt"] = powt

        def stage_smv(b):
            a_all, xh = S[b]["a_all"], S[b]["xh"]
            if b == 0:
                S[0]["s_sh"] = ps_s.tile([128, 2, NNB], f32, tag="s",
                                         name="s_sh")
                S[1]["s_sh"] = S[0]["s_sh"]
            s_ps = S[b]["s_sh"][:, b, :]
            for a in range(NCH):
                xt = xh[a // 2]
                for jb in range(NNB):
                    lhsT = bass.AP(
                        tensor=xt.tensor,
                        offset=xt.offset + (a % 2) * N + jb,
                        ap=[list(xt.ap[0]), [NNB, 128]])
                    nc.tensor.matmul(s_ps[:, jb:jb + 1], lhsT,
                                     a_all[:, a:a + 1], start=(a == 0),
                                     stop=(a == NCH - 1),
                                     skip_group_check=True)
            S[b]["s_ps"] = s_ps

        def stage_mom(b):
            powt = S[b]["powt"]
            mom_ps = ps_sm.tile([1, NS * (NFC + 1)], f32, tag="sm",
                                name=f"mom{b}")
            nc.tensor.matmul(mom_ps, ones_col,
                             powt.rearrange("p s a -> p (s a)"),
                             start=True, stop=True)
            coefs = bpool.tile([1, NS], f32, tag="coefs", name=f"coefs{b}")
            nc.vector.tensor_reduce(
                coefs, mom_ps.rearrange("o (s a) -> o s a", s=NS),
                axis=Ax.X, op=Alu.add)
            crep_ps = ps_sm.tile([128, NS], f32, tag="sm", name=f"crep{b}")
            nc.tensor.matmul(crep_ps, ones_row, coefs, start=True, stop=True)
            crep = bpool.tile([128, NS], f32, tag="crep", name=f"crep{b}")
            nc.vector.tensor_mul(crep, crep_ps, invf)
            S[b]["crep"] = crep

        def stage_eval(b):
            s_ps, constb, crep = S[b]["s_ps"], S[b]["constb"], S[b]["crep"]
            # slot-major storage: spow[:, j, :] = s^(j+1); every engine WRITE
            # is innermost-contiguous (strided-innermost outputs fail the
            # walrus BIR verifier); the dot reads it via a transposed AP.
            spow = bpool.tile([128, J, NNB + 1], f32, tag="spow",
                              name=f"spow{b}")
            nc.vector.tensor_scalar_add(spow[:, 0, 0:NNB], s_ps, constb)

            def sbc(j0, brep):
                a = spow[:, j0, 0:NNB]
                return bass.AP(tensor=a.tensor, offset=a.offset,
                               ap=[list(a.ap[0]), [0, brep], [1, NNB]])
            nc.gpsimd.tensor_mul(spow[:, 1, 0:NNB], spow[:, 0, 0:NNB],
                                 spow[:, 0, 0:NNB])
            nc.gpsimd.tensor_tensor(spow[:, 2:4, 0:NNB], spow[:, 0:2, 0:NNB],
                                    sbc(1, 2), Alu.mult)
            nc.gpsimd.tensor_tensor(spow[:, 4:8, 0:NNB], spow[:, 0:4, 0:NNB],
                                    sbc(3, 4), Alu.mult)
            spow_nm = bass.AP(tensor=spow.tensor, offset=spow.offset,
                              ap=[list(spow.ap[0]), [1, NNB], [NNB + 1, J]])
            pp = bpool.tile([128, 2, NNB, J + 1], f32, tag="pp",
                            name=f"pp{b}")
            dn = bpool.tile([128, 2, NNB], f32, tag="dn", name=f"dn{b}")

            def dot(which, eng, ccols, c0_ap, c0_imm):
                cb = bass.AP(tensor=crep.tensor,
                             offset=crep.offset + ccols,
                             ap=[list(crep.ap[0]), [0, NNB], [1, J]])
                eng.tensor_tensor(pp[:, which, :, 0:J], spow_nm, cb, Alu.mult)
                nc.vector.tensor_reduce(dn[:, which, :],
                                        pp[:, which, :, 0:J],
                                        axis=Ax.X, op=Alu.add)
                if c0_ap is not None:
                    eng.tensor_scalar_add(dn[:, which, :],
                                          dn[:, which, :], c0_ap)
                else:
                    nc.vector.tensor_scalar_add(dn[:, which, :],
                                                dn[:, which, :], c0_imm)

            dot(1, nc.vector, 0, None, float(F))           # denominator
            dot(0, nc.gpsimd if b == 0 else nc.vector,
                J + 1, crep[:, J:J + 1], None)             # numerator
            rd = bpool.tile([128, NNB], f32, tag="rd", name=f"rd{b}")
            nc.vector.reciprocal(rd, dn[:, 1, :])
            w_nm = bpool.tile([128, NNB], bf16, tag="w_nm", name=f"w{b}")
            nc.vector.tensor_mul(w_nm, dn[:, 0, :], rd)
            # scatter n-major w back to a row: w2[0, n] = w_nm[n//8, n%8]
            # (DMA pairs elements in flatten order; 16B descriptors)
            rings[b].dma_start(w2t[b][0:1, :], w_nm)
            S[b]["w2"] = w2t[b]

        def stage_out(b):
            xh, w2 = S[b]["xh"], S[b]["w2"]
            o_sb = opool.tile([128, NCH, N], bf16, tag="o", name=f"o{b}")
            for a in range(NCH):
                for h in range(2):
                    dve = (h == 1)
                    o_ps = ps_out.tile([128, 512], f32, tag="o",
                                       name=f"ops{b}{a}{h}")
                    nc.tensor.matmul(o_ps, wobo[:, a, :],
                                     w2[:, 512 * h:512 * (h + 1)],
                                     start=True, stop=dve,
                                     skip_group_check=True)
                    xa = xh[a // 2][:, a % 2, :]
                    dst = o_sb[:, a, 512 * h:512 * (h + 1)]
                    if dve:
                        nc.vector.tensor_add(
                            dst, o_ps, xa[:, 512 * h:512 * (h + 1)])
                    else:
                        nc.tensor.matmul(o_ps, ident,
                                         xa[:, 512 * h:512 * (h + 1)],
                                         start=False, stop=True,
                                         skip_group_check=True)
                        nc.scalar.copy(dst, o_ps)
                rings[b].dma_start(out_d[b, 128 * a:128 * (a + 1), :],
                                   o_sb[:, a, :])

        def emit_rep():
            stage_load(0)
            stage_load(1)
            if S[0].get("first", True):
                emit_prologue()
                S[0]["first"] = False
            stage_stats(0)
            stage_kv(0)
            stage_smv(0)
            stage_mom(0)
            stage_eval(0)
            stage_out(0)
            stage_stats(1)
            stage_kv(1)
            stage_smv(1)
            stage_mom(1)
            stage_eval(1)
            stage_out(1)

        if loops == 1:
            for _ in range(reps):
                emit_rep()
        else:
            emit_prologue()
            S[0]["first"] = False
            with tc.For_i(0, loops):
                for _ in range(reps):
                    emit_rep()

    if legalize:
        _legalize_sync(nc, mybir)
    return nc


def _indicators():
    ind128 = np.zeros((128, 8), np.float32)
    indT8 = np.zeros((8, 128), np.float32)
    for g in range(8):
        ind128[16 * g:16 * g + 16, g] = 1.0 / 16.0
        indT8[g, 16 * g:16 * g + 16] = 1.0
    return ind128, indT8


def _host_pack(inputs):
    """Weight-only prepacking + dtype/layout prep shared by all cores."""
    import ml_dtypes
    import math
    bf = ml_dtypes.bfloat16
    f = {k: np.asarray(v, dtype=np.float32) for k, v in inputs.items()}
    wqc = f["wq"].sum(axis=0)                       # colsum over out channels
    wg = (wqc * f["gamma"]).astype(np.float32)      # [C]
    sconst = np.float32(f["bq"].sum() + (wqc * f["beta"]).sum())
    wkTs = (f["wk"].T * SCALE).astype(bf)           # [T, F]
    wvT = f["wv"].T.astype(bf)
    bks = (f["bk"] * SCALE).astype(np.float32)
    bkv = np.stack([bks.reshape(NFC, 128).T,
                    f["bv"].astype(np.float32).reshape(NFC, 128).T],
                   axis=1)                          # [128, 2, NFC]
    wos = f["wo"].sum(axis=1)
    wobo = np.stack([wos.reshape(NCH, 128),
                     f["bo"].reshape(NCH, 128)], axis=0).astype(bf)
    invf = np.array([1.0 / math.factorial(j) for j in range(1, J + 1)] +
                    [1.0 / math.factorial(j) for j in range(J + 1)],
                    np.float32)
    ind128, indT8 = _indicators()

    p128 = np.zeros((128, 46), np.float32)
    p128[:, 0:8] = ind128
    p128[:, 8:12] = wg.reshape(NCH, 128).T
    p128[:, 12] = sconst
    p128[:, 13:29] = bkv.reshape(128, 16)
    p128[:, 29:46] = invf[None, :]
    p8 = np.zeros((8, 132), np.float32)
    p8[:, 0:128] = indT8
    p8[:, 128:132] = wg.reshape(G, C // G).sum(1).reshape(NCH, 8).T
    pbf = np.zeros((128, 2176), bf)
    pbf[:, 0:128] = np.eye(128, dtype=bf)
    pbf[:, 128:1152] = wkTs[0:128]
    pbf[:, 1152:2176] = wvT[0:128]
    p52 = np.zeros((T - 128, 2 * F), bf)
    p52[:, 0:F] = wkTs[128:T]
    p52[:, F:2 * F] = wvT[128:T]
    shared = {
        "p128f": p128, "p8f": p8, "pbf": pbf, "p52": p52,
        "wobo": np.ascontiguousarray(wobo),
    }
    x_bf = np.asarray(f["x"], np.float32).reshape(B, C, N).astype(bf)
    condp = np.zeros((128, 2 * B), bf)
    condT = f["condition"].T.astype(bf)             # [T, B]
    for b in range(B):
        condp[:, 2 * b] = condT[0:128, b]
        condp[0:T - 128, 2 * b + 1] = condT[128:T, b]
    return shared, x_bf, condp


def _dev_inputs(inputs, core_idx, _packed=None):
    shared, x_bf, x_bf_cond = (_packed if _packed is not None
                               else _host_pack(inputs))
    m = dict(shared)
    m["x_sh"] = np.ascontiguousarray(x_bf[BPC * core_idx:BPC * (core_idx + 1)])
    m["condp"] = np.ascontiguousarray(
        x_bf_cond[:, 2 * BPC * core_idx:2 * BPC * (core_idx + 1)])
    return m


def kernel(**inputs):
    from concourse.bass_utils import run_bass_kernel_spmd

    if "nc" not in _CACHE:
        _CACHE["nc"] = _build()
    nc = _CACHE["nc"]

    packed = _host_pack(inputs)
    in_maps = [_dev_inputs(inputs, i, packed) for i in range(NCORES)]
    res = run_bass_kernel_spmd(nc, in_maps, core_ids=list(range(NCORES)))
    out = np.concatenate([np.asarray(r["out"], dtype=np.float32)
                          for r in res.results], axis=0)
    return out.reshape(B, C, HW, HW)
